# revision 1
# baseline (speedup 1.0000x reference)
"""Trainium2 Bass kernel: 3x3 stride-1 pad-1 Conv2d, 16->16 channels, 1024x1024.

Strategy (8 NeuronCores, spatial split over H):
  - Core i computes output rows [128*i, 128*i+128). Its input slice holds rows
    128*i-1 .. 128*i+132 (1-row halo + zero padding), pre-padded on the host with
    one zero column on each side so horizontal taps are pure free-dim shifts.
  - Inside a core: groups of 6 output rows. A group's rhs is one SBUF tile of
    [128 partitions = (row r 0..7) x (channel c 0..15), 1032 free] holding the
    8 input rows the 6 outputs need. The 3x3 conv becomes 3 accumulating
    matmuls (one per horizontal tap kw) against a block-banded [128,128]
    weight matrix: lhsT[(r,c),(g,o)] = W[o,c,kh=r-g,kw] for 0<=r-g<=2, g<6.
    Vertical taps live in the (r,g) band structure; horizontal taps are
    free-dim offsets of the rhs slice. fp32r matmul dtype: ~1 cycle/row on PE
    with ~1e-4 relative error (measured), accumulation in fp32 PSUM.
  - PSUM [128,512] per half-window -> copy to SBUF staging -> DMA to DRAM
    output laid out [row, c_out, w] per core (host reassembles).
"""

import sys

sys.path.insert(0, "/opt/trn_rl_repo")

import numpy as np

import concourse.bass as bass  # noqa: F401  (engine handles live on nc)
import concourse.mybir as mybir
import concourse.tile as tile
from concourse import bacc
from concourse.bass_utils import run_bass_kernel_spmd

C = 16          # channels in/out
H = 1024        # image height/width
W = 1024
NCORES = 8
RPC = H // NCORES       # output rows per core = 128
ADV = 6                 # output rows per group
GROUPS = (RPC + ADV - 1) // ADV   # 22 groups (last partial: 2 rows)
SROWS = ADV * (GROUPS - 1) + 8    # input slice rows needed = 134
WPAD = 1032             # padded row width (col 0 and 1025 are zeros, 1..1024 data)
NVALID = 1026           # columns actually read per row

_CACHE = {}


def _build_nc(reps: int = 1, halo_sb: bool = False, in_dt: str = "float16",
              bufs=(8, 6, 6), taps: int = 3):
    key = ("nc", reps, halo_sb, in_dt, bufs, taps)
    if key in _CACHE:
        return _CACHE[key]
    nc = bacc.Bacc("TRN2", target_bir_lowering=False, debug=False)
    f32 = mybir.dt.float32
    f32r = getattr(mybir.dt, in_dt)
    xs = nc.dram_tensor("xs", [SROWS, C, WPAD], f32r, kind="ExternalInput").ap()
    wpk = nc.dram_tensor("wpk", [128, 3 * 128], f32r, kind="ExternalInput").ap()
    out = nc.dram_tensor("out", [RPC, C, W], f32, kind="ExternalOutput").ap()

    with tile.TileContext(nc) as tc:
        with (
            tc.tile_pool(name="wp", bufs=1) as wp,
            tc.tile_pool(name="xin", bufs=bufs[0]) as xin,
            tc.tile_pool(name="ps", bufs=bufs[1], space="PSUM") as ps,
            tc.tile_pool(name="ost", bufs=bufs[2]) as ostp,
        ):
            wt = wp.tile([128, 3 * 128], f32r)
            nc.sync.dma_start(out=wt, in_=wpk)
            xsf = xs.flatten_outer_dims()  # [SROWS*C, WPAD]
            of = out.flatten_outer_dims()  # [RPC*C, W]

            def body(_i=None):
                prev = None
                for t in range(GROUPS):
                    xt = xin.tile([128, WPAD], f32r)
                    if halo_sb and prev is not None:
                        # overlap rows 6t..6t+1 come from the previous tile's
                        # partitions 96..127 (SBUF->SBUF), only 6 new rows
                        # from HBM
                        nc.sync.dma_start(out=xt[0:32], in_=prev[96:128])
                        nc.sync.dma_start(
                            out=xt[32:128],
                            in_=xsf[ADV * C * t + 32 : ADV * C * t + 128],
                        )
                    else:
                        nc.sync.dma_start(
                            out=xt,
                            in_=xsf[ADV * C * t : ADV * C * t + 128],
                        )
                    prev = xt
                    ost = ostp.tile([128, W], f32)
                    for h in range(2):
                        pt = ps.tile([128, 512], f32)
                        for kw in range(taps):
                            nc.tensor.matmul(
                                pt,
                                wt[:, kw * 128 : (kw + 1) * 128],
                                xt[:, h * 512 + kw : h * 512 + kw + 512],
                                start=(kw == 0),
                                stop=(kw == taps - 1),
                            )
                        nc.vector.tensor_copy(ost[:, h * 512 : (h + 1) * 512], pt)
                    rows = min(ADV, RPC - ADV * t)
                    # out-DMA on the ACT HWDGE ring: its sem-wait (on the
                    # PSUM copies) must not head-of-line-block the next input
                    # DMA, which stays on the SP ring.
                    nc.scalar.dma_start(
                        out=of[ADV * C * t : ADV * C * t + rows * C],
                        in_=ost[0 : rows * C],
                    )

            if reps > 1:
                with tc.For_i(0, reps, 1) as _i:
                    body(_i)
            else:
                body()
    nc.compile()
    _CACHE[key] = nc
    return nc


def _pack_weights(weight: np.ndarray) -> np.ndarray:
    """wpk[(r*16+c), kw*128 + (g*16+o)] = W[o,c,r-g,kw] for 0<=r-g<=2, g<6."""
    wpk = np.zeros((8, C, 3, 8, C), dtype=np.float32)  # [r, c, kw, g, o]
    wt = weight.astype(np.float32).transpose(1, 3, 0, 2)  # [c, kw, o, kh]
    for g in range(ADV):
        for kh in range(3):
            wpk[g + kh, :, :, g, :] = wt[:, :, :, kh]
    return np.ascontiguousarray(wpk.reshape(128, 3 * 128))


def _slice_inputs(x: np.ndarray) -> list[np.ndarray]:
    """Per-core input slices [SROWS, C, WPAD], row-major, zero-padded."""
    xr = x[0].transpose(1, 0, 2)  # [H, C, W]
    gpad = np.zeros((NCORES * RPC + SROWS, C, WPAD), dtype=np.float32)
    gpad[1 : H + 1, :, 1 : W + 1] = xr
    return [np.ascontiguousarray(gpad[RPC * i : RPC * i + SROWS]) for i in range(NCORES)]


def kernel(x: np.ndarray, weight: np.ndarray, _run_kw: dict | None = None):
    nc = _build_nc()
    wpk = _pack_weights(weight).astype(np.float16)
    slices = [s.astype(np.float16) for s in _slice_inputs(np.asarray(x, dtype=np.float32))]
    in_maps = [{"xs": s, "wpk": wpk} for s in slices]
    res = run_bass_kernel_spmd(
        nc, in_maps, core_ids=list(range(NCORES)), **(_run_kw or {})
    )
    outs = np.stack([res.results[i]["out"] for i in range(NCORES)])  # [i, g, o, w]
    full = outs.transpose(2, 0, 1, 3).reshape(C, H, W)
    if _run_kw:
        kernel.last_results = res
    return full



# revision 3
# speedup vs baseline: 1.0559x; 1.0559x over previous
"""Trainium2 Bass kernel: 3x3 stride-1 pad-1 Conv2d, 16->16 channels, 1024x1024.

Strategy (8 NeuronCores, spatial split over H):
  - Core i computes output rows [128*i, 128*i+128). Its input slice holds rows
    128*i-1 .. 128*i+132 (1-row halo + zero padding), pre-padded on the host with
    one zero column on each side so horizontal taps are pure free-dim shifts.
  - Inside a core: groups of 6 output rows. A group's rhs is one SBUF tile of
    [128 partitions = (row r 0..7) x (channel c 0..15), 1032 free] holding the
    8 input rows the 6 outputs need. The 3x3 conv becomes 3 accumulating
    matmuls (one per horizontal tap kw) against a block-banded [128,128]
    weight matrix: lhsT[(r,c),(g,o)] = W[o,c,kh=r-g,kw] for 0<=r-g<=2, g<6.
    Vertical taps live in the (r,g) band structure; horizontal taps are
    free-dim offsets of the rhs slice. fp16 matmul dtype: 1 cycle/row on PE,
    accumulation in fp32 PSUM.
  - Each group accumulates into a [128,1024] fp32 PSUM tile (2 banks; each
    matmul targets one bank), then ONE PSUM->SBUF copy converts to fp16,
    alternating vector/scalar engines per group. Output DMA (fp16) goes out
    on the gpsimd SWDGE ring so neither HWDGE ring head-of-line-blocks.
  - halo reuse: only 96 of 128 input partitions come from HBM per group; the
    2-row overlap is an SBUF->SBUF copy from the previous tile.
"""

import sys

sys.path.insert(0, "/opt/trn_rl_repo")

import numpy as np

import concourse.bass as bass  # noqa: F401  (engine handles live on nc)
import concourse.mybir as mybir
import concourse.tile as tile
from concourse import bacc
from concourse.bass_utils import run_bass_kernel_spmd

C = 16          # channels in/out
H = 1024        # image height/width
W = 1024
NCORES = 8
RPC = H // NCORES       # output rows per core = 128
ADV = 6                 # output rows per group
GROUPS = (RPC + ADV - 1) // ADV   # 22 groups (last partial: 2 rows)
SROWS = ADV * (GROUPS - 1) + 8    # input slice rows needed = 134
WPAD = 1032             # padded row width (col 0 and 1025 are zeros, 1..1024 data)
NVALID = 1026           # columns actually read per row

_CACHE = {}


def _build_nc(reps: int = 1, halo_sb: bool = False, in_dt: str = "float16",
              out_dt: str = "float16", bufs=(8, 3, 6), taps: int = 3,
              out_eng: str = "gpsimd", copy_split: str = "vs"):
    key = ("nc", reps, halo_sb, in_dt, out_dt, bufs, taps, out_eng, copy_split)
    if key in _CACHE:
        return _CACHE[key]
    nc = bacc.Bacc("TRN2", target_bir_lowering=False, debug=False)
    f32 = mybir.dt.float32
    fin = getattr(mybir.dt, in_dt)
    fout = getattr(mybir.dt, out_dt)
    xs = nc.dram_tensor("xs", [SROWS, C, WPAD], fin, kind="ExternalInput").ap()
    wpk = nc.dram_tensor("wpk", [128, 3 * 128], fin, kind="ExternalInput").ap()
    out = nc.dram_tensor("out", [RPC, C, W], fout, kind="ExternalOutput").ap()

    with tile.TileContext(nc) as tc:
        with (
            tc.tile_pool(name="wp", bufs=1) as wp,
            tc.tile_pool(name="xin", bufs=bufs[0]) as xin,
            tc.tile_pool(name="ps", bufs=bufs[1], space="PSUM") as ps,
            tc.tile_pool(name="ost", bufs=bufs[2]) as ostp,
        ):
            wt = wp.tile([128, 3 * 128], fin)
            # weight load on the ACT HWDGE ring keeps the SP ring free for
            # the first input tile
            nc.scalar.dma_start(out=wt, in_=wpk)
            xsf = xs.flatten_outer_dims()  # [SROWS*C, WPAD]
            of = out.flatten_outer_dims()  # [RPC*C, W]

            def body(_i=None):
                prev = None
                for t in range(GROUPS):
                    xt = xin.tile([128, WPAD], fin)
                    if halo_sb and prev is not None:
                        # overlap rows 6t..6t+1 come from the previous tile's
                        # partitions 96..127 (SBUF->SBUF), only 6 new rows
                        # from HBM
                        nc.sync.dma_start(out=xt[0:32], in_=prev[96:128])
                        nc.sync.dma_start(
                            out=xt[32:128],
                            in_=xsf[ADV * C * t + 32 : ADV * C * t + 128],
                        )
                    else:
                        nc.sync.dma_start(
                            out=xt,
                            in_=xsf[ADV * C * t : ADV * C * t + 128],
                        )
                    prev = xt
                    pt = ps.tile([128, 1024], f32)  # 2 PSUM banks
                    for kw in range(taps):
                        for h in range(2):
                            nc.tensor.matmul(
                                pt[:, h * 512 : (h + 1) * 512],
                                wt[:, kw * 128 : (kw + 1) * 128],
                                xt[:, h * 512 + kw : h * 512 + kw + 512],
                                start=(kw == 0),
                                stop=(kw == taps - 1),
                            )
                    ost = ostp.tile([128, W], fout)
                    # one wide PSUM->SBUF copy (converts to fp16), alternating
                    # engines so neither becomes the bottleneck
                    if copy_split == "vs":
                        if t % 2 == 0:
                            nc.vector.tensor_copy(ost, pt)
                        else:
                            nc.scalar.copy(ost, pt)
                    elif copy_split == "v":
                        nc.vector.tensor_copy(ost, pt)
                    else:
                        raise ValueError(copy_split)
                    rows = min(ADV, RPC - ADV * t)
                    oeng = getattr(nc, out_eng)
                    oeng.dma_start(
                        out=of[ADV * C * t : ADV * C * t + rows * C],
                        in_=ost[0 : rows * C],
                    )

            if reps > 1:
                with tc.For_i(0, reps, 1) as _i:
                    body(_i)
            else:
                body()
    nc.compile()
    _CACHE[key] = nc
    return nc


def _pack_weights(weight: np.ndarray) -> np.ndarray:
    """wpk[(r*16+c), kw*128 + (g*16+o)] = W[o,c,r-g,kw] for 0<=r-g<=2, g<6."""
    wpk = np.zeros((8, C, 3, 8, C), dtype=np.float32)  # [r, c, kw, g, o]
    wt = weight.astype(np.float32).transpose(1, 3, 0, 2)  # [c, kw, o, kh]
    for g in range(ADV):
        for kh in range(3):
            wpk[g + kh, :, :, g, :] = wt[:, :, :, kh]
    return np.ascontiguousarray(wpk.reshape(128, 3 * 128))


def _slice_inputs(x: np.ndarray) -> list[np.ndarray]:
    """Per-core input slices [SROWS, C, WPAD], row-major, zero-padded."""
    xr = x[0].transpose(1, 0, 2)  # [H, C, W]
    gpad = np.zeros((NCORES * RPC + SROWS, C, WPAD), dtype=np.float32)
    gpad[1 : H + 1, :, 1 : W + 1] = xr
    return [np.ascontiguousarray(gpad[RPC * i : RPC * i + SROWS]) for i in range(NCORES)]


def kernel(x: np.ndarray, weight: np.ndarray, _run_kw: dict | None = None,
           _build_kw: dict | None = None):
    nc = _build_nc(**(_build_kw or {}))
    wpk = _pack_weights(weight).astype(np.float16)
    slices = [s.astype(np.float16) for s in _slice_inputs(np.asarray(x, dtype=np.float32))]
    in_maps = [{"xs": s, "wpk": wpk} for s in slices]
    res = run_bass_kernel_spmd(
        nc, in_maps, core_ids=list(range(NCORES)), **(_run_kw or {})
    )
    outs = np.stack([res.results[i]["out"] for i in range(NCORES)])  # [i, g, o, w]
    full = outs.transpose(2, 0, 1, 3).reshape(C, H, W).astype(np.float32)
    if _run_kw:
        kernel.last_results = res
    return full


# revision 7
# speedup vs baseline: 1.2887x; 1.2205x over previous
"""Trainium2 Bass kernel: 3x3 stride-1 pad-1 Conv2d, 16->16 channels, 1024x1024.

Strategy (8 NeuronCores, spatial split over H):
  - Core i computes output rows [128*i, 128*i+128). Its input slice holds rows
    128*i-1 .. 128*i+132 (1-row halo + zero padding), pre-padded on the host with
    one zero column on each side so horizontal taps are pure free-dim shifts.
  - Inside a core: groups of 6 output rows. A group's rhs is one SBUF tile of
    [128 partitions = (row r 0..7) x (channel c 0..15), 1032 free] holding the
    8 input rows the 6 outputs need. The 3x3 conv becomes 3 accumulating
    matmuls (one per horizontal tap kw) against a block-banded [128,128]
    weight matrix: lhsT[(r,c),(g,o)] = W[o,c,kh=r-g,kw] for 0<=r-g<=2, g<6.
    Vertical taps live in the (r,g) band structure; horizontal taps are
    free-dim offsets of the rhs slice. fp16 matmul dtype: 1 cycle/row on PE,
    accumulation in fp32 PSUM.
  - Each group accumulates into a [128,1024] fp32 PSUM tile (2 banks; each
    matmul targets one bank), then ONE PSUM->SBUF copy converts to fp16,
    alternating vector/scalar engines per group. Output DMA (fp16) goes out
    on the gpsimd SWDGE ring so neither HWDGE ring head-of-line-blocks.
  - halo reuse: only 96 of 128 input partitions come from HBM per group; the
    2-row overlap is an SBUF->SBUF copy from the previous tile.
"""

import sys

sys.path.insert(0, "/opt/trn_rl_repo")

import numpy as np

import concourse.bass as bass  # noqa: F401  (engine handles live on nc)
import concourse.mybir as mybir
import concourse.tile as tile
from concourse import bacc
from concourse.bass_utils import run_bass_kernel_spmd

C = 16          # channels in/out
H = 1024        # image height/width
W = 1024
NCORES = 8
RPC = H // NCORES       # output rows per core = 128
ADV = 6                 # output rows per group
GROUPS = (RPC + ADV - 1) // ADV   # 22 groups (last partial: 2 rows)
SROWS = ADV * (GROUPS - 1) + 8    # input slice rows needed = 134
WPAD = 1032             # padded row width (col 0 and 1025 are zeros, 1..1024 data)
NVALID = 1026           # columns actually read per row

_CACHE = {}


def _build_nc(reps: int = 1, in_dt: str = "float16",
              out_dt: str = "float16", bufs=(8, 3, 6), taps: int = 3,
              out_eng: str = "gpsimd", warmup: int = 22, pair: bool = True):
    key = ("nc", reps, in_dt, out_dt, bufs, taps, out_eng, warmup, pair)
    if key in _CACHE:
        return _CACHE[key]
    nc = bacc.Bacc("TRN2", target_bir_lowering=False, debug=False)
    f32 = mybir.dt.float32
    fin = getattr(mybir.dt, in_dt)
    fout = getattr(mybir.dt, out_dt)
    xs = nc.dram_tensor("xs", [SROWS, C, WPAD], fin, kind="ExternalInput").ap()
    wpk = nc.dram_tensor("wpk", [128, 3 * 128], fin, kind="ExternalInput").ap()
    out = nc.dram_tensor("out", [RPC, C, W], fout, kind="ExternalOutput").ap()

    with tile.TileContext(nc) as tc:
        with (
            tc.tile_pool(name="wp", bufs=1) as wp,
            tc.tile_pool(name="xin", bufs=bufs[0]) as xin,
            tc.tile_pool(name="ps", bufs=bufs[1], space="PSUM") as ps,
            tc.tile_pool(name="ost", bufs=bufs[2]) as ostp,
            tc.tile_pool(name="wups", bufs=1 if warmup else 0, space="PSUM") as wups,
        ):
            wt = wp.tile([128, 3 * 128], fin)
            # weight load on the ACT HWDGE ring keeps the SP ring free for
            # the first input tile
            nc.scalar.dma_start(out=wt, in_=wpk)
            xsf = xs.flatten_outer_dims()  # [SROWS*C, WPAD]
            of = out.flatten_outer_dims()  # [RPC*C, W]

            if warmup:
                # HAM warm-up: keep the PE busy ~3.5us on junk data while the
                # first input tiles stream in, so the real matmuls start at
                # K=8/8 (2.4 GHz) instead of ramping through 1.2 GHz.
                wux = wp.tile([128, 256], fin)
                nc.gpsimd.memset(wux, 0.0)
                wup = wups.tile([128, 192], f32)
                for _ in range(warmup):
                    nc.tensor.matmul(wup, wux[:, :128], wux[:, 64 : 64 + 192],
                                     start=True, stop=True)

            def group_mms(pt, xt, kw):
                for h in range(2):
                    nc.tensor.matmul(
                        pt[:, h * 512 : (h + 1) * 512],
                        wt[:, kw * 128 : (kw + 1) * 128],
                        xt[:, h * 512 + kw : h * 512 + kw + 512],
                        start=(kw == 0),
                        stop=(kw == taps - 1),
                    )

            def finish_group(t, pt, copy_vec):
                ost = ostp.tile([128, W], fout)
                if copy_vec:
                    nc.vector.tensor_copy(ost, pt)
                else:
                    nc.scalar.copy(ost, pt)
                rows = min(ADV, RPC - ADV * t)
                getattr(nc, out_eng).dma_start(
                    out=of[ADV * C * t : ADV * C * t + rows * C],
                    in_=ost[0 : rows * C],
                )

            def body(_i=None):
                if pair:
                    for t2 in range((GROUPS + 1) // 2):
                        ts = [t for t in (2 * t2, 2 * t2 + 1) if t < GROUPS]
                        xts, pts = [], []
                        for t in ts:
                            xt = xin.tile([128, WPAD], fin, name="xt")
                            nc.sync.dma_start(
                                out=xt, in_=xsf[ADV * C * t : ADV * C * t + 128])
                            xts.append(xt)
                            pts.append(ps.tile([128, 1024], f32, name="pt"))
                        # 4 matmuls per weight load: kw outer, (group, half) inner
                        for kw in range(taps):
                            for pt, xt in zip(pts, xts):
                                group_mms(pt, xt, kw)
                        for j, t in enumerate(ts):
                            finish_group(t, pts[j], copy_vec=(j == 0))
                else:
                    for t in range(GROUPS):
                        xt = xin.tile([128, WPAD], fin)
                        nc.sync.dma_start(
                            out=xt, in_=xsf[ADV * C * t : ADV * C * t + 128])
                        pt = ps.tile([128, 1024], f32)
                        for kw in range(taps):
                            group_mms(pt, xt, kw)
                        finish_group(t, pt, copy_vec=(t % 2 == 0))

            if reps > 1:
                with tc.For_i(0, reps, 1) as _i:
                    body(_i)
            else:
                body()
    nc.compile()
    _CACHE[key] = nc
    return nc


def _pack_weights(weight: np.ndarray) -> np.ndarray:
    """wpk[(r*16+c), kw*128 + (g*16+o)] = W[o,c,r-g,kw] for 0<=r-g<=2, g<6."""
    wpk = np.zeros((8, C, 3, 8, C), dtype=np.float32)  # [r, c, kw, g, o]
    wt = weight.astype(np.float32).transpose(1, 3, 0, 2)  # [c, kw, o, kh]
    for g in range(ADV):
        for kh in range(3):
            wpk[g + kh, :, :, g, :] = wt[:, :, :, kh]
    return np.ascontiguousarray(wpk.reshape(128, 3 * 128))


def _slice_inputs(x: np.ndarray) -> list[np.ndarray]:
    """Per-core input slices [SROWS, C, WPAD], row-major, zero-padded."""
    xr = x[0].transpose(1, 0, 2)  # [H, C, W]
    gpad = np.zeros((NCORES * RPC + SROWS, C, WPAD), dtype=np.float32)
    gpad[1 : H + 1, :, 1 : W + 1] = xr
    return [np.ascontiguousarray(gpad[RPC * i : RPC * i + SROWS]) for i in range(NCORES)]


def kernel(x: np.ndarray, weight: np.ndarray, _run_kw: dict | None = None,
           _build_kw: dict | None = None):
    nc = _build_nc(**(_build_kw or {}))
    wpk = _pack_weights(weight).astype(np.float16)
    slices = [s.astype(np.float16) for s in _slice_inputs(np.asarray(x, dtype=np.float32))]
    in_maps = [{"xs": s, "wpk": wpk} for s in slices]
    res = run_bass_kernel_spmd(
        nc, in_maps, core_ids=list(range(NCORES)), **(_run_kw or {})
    )
    outs = np.stack([res.results[i]["out"] for i in range(NCORES)])  # [i, g, o, w]
    full = outs.transpose(2, 0, 1, 3).reshape(C, H, W).astype(np.float32)
    if _run_kw:
        kernel.last_results = res
    return full
